# revision 1
# baseline (speedup 1.0000x reference)
"""Causal self-attention Trainium2 Bass kernel.

Problem (hardcoded): x [4, 2048, 1024] f32, wq/wk/wv/wo [1024, 1024], biases
[1024]; out = causal_mha(x) @ wo + bo with 16 heads of dim 64.

Sharding over 8 NeuronCores: data parallel on batch (4) x tensor parallel on
heads (2 groups of 8 heads). Core c handles batch c//2 and head-group c%2.
Each core computes its partial out-projection (its 8 heads through its rows of
wo); the host sums the two partials per batch and adds the bias terms
(bo + bv @ wo, since softmax rows sum to 1 the v-bias contributes exactly
bv @ wo).

Device pipeline per core (all matmuls in fp32r: fp32 operands truncated to
FP22 on read, fp32 PSUM accumulation, full tensor-engine rate):
  phase 1: qT/kT = (wq/wk)^T @ x^T (+bias), v = x @ wv, streaming x^T chunks
  phase 2: per q-chunk of 512 and head: ST[k,q] = k^T q blocks, additive
    causal mask on diagonal blocks, P = exp(0.125*ST) (ScalarE), unnormalized
    O^T = [v|1]^T @ P via PSUM accumulation (ones column yields softmax sums),
    normalization by 1/sum broadcast via a K=1 matmul, then the out-projection
    out = O^T.T @ wo from the transposed attention output.
"""

import numpy as np

N_HEADS = 16
DH = 64
N_CORES = 8
TP = 2  # head groups

_cache = {}
TRACE = False  # set by test harness to request an NTFF trace
last_result = None  # BassKernelResults of the most recent kernel() call


def _build(SEQ, D, DG, HPG, reps=1):
    """Build + schedule the per-core Bass program. DG = per-core qkv width,
    HPG = heads per core."""
    from contextlib import ExitStack

    import concourse.tile as tile
    from concourse import bacc, mybir

    F32 = mybir.dt.float32
    F32R = mybir.dt.float32r
    AF = mybir.ActivationFunctionType
    ALU = mybir.AluOpType

    KO = D // 128  # contraction subtiles for the projections
    MQ = DG // 128  # qkv-dim subtiles
    SC = 512  # q/s chunk size
    NSC = SEQ // SC  # chunks
    NJ = SC // 128  # 128-blocks per chunk
    NSB = SEQ // 128  # s blocks total
    NO = D // 512  # out-proj column chunks

    nc = bacc.Bacc("TRN2", target_bir_lowering=False, debug=False)
    xT = nc.dram_tensor("xT", [D, SEQ], F32R, kind="ExternalInput")
    wq = nc.dram_tensor("wq", [D, DG], F32R, kind="ExternalInput")
    wk = nc.dram_tensor("wk", [D, DG], F32R, kind="ExternalInput")
    wv = nc.dram_tensor("wv", [D, DG], F32R, kind="ExternalInput")
    wo = nc.dram_tensor("wo", [DG, D], F32R, kind="ExternalInput")
    bq = nc.dram_tensor("bq", [DG], F32, kind="ExternalInput")
    bk = nc.dram_tensor("bk", [DG], F32, kind="ExternalInput")
    out = nc.dram_tensor("out", [SEQ, D], F32, kind="ExternalOutput")

    scale = 1.0 / np.sqrt(DH)

    rep_range = range(reps)
    with tile.TileContext(nc) as tc, ExitStack() as ctx:
        # pools alive for the whole kernel
        res = ctx.enter_context(tc.tile_pool(name="res", bufs=1))
        qT = res.tile([128, MQ, SEQ], F32R, tag="qT", name="qT")
        kT = res.tile([128, MQ, SEQ], F32R, tag="kT", name="kT")
        vn = res.tile([128, NSB, HPG, DH + 1], F32R, tag="vn", name="vn")

        ones64 = res.tile([1, 64], F32R, tag="ones64", name="ones64")
        ones64_f = res.tile([1, 64], F32, tag="ones64_f", name="ones64_f")
        nc.gpsimd.memset(ones64_f[:], 1.0)
        nc.vector.tensor_copy(ones64[:], ones64_f[:])
        ones_nb = res.tile([128, NSB, HPG], F32, tag="ones_nb", name="ones_nb")
        nc.gpsimd.memset(ones_nb[:], 1.0)
        nc.vector.tensor_copy(vn[:, :, :, DH], ones_nb[:])

        bq_sb = res.tile([128, MQ], F32, tag="bq_sb", name="bq_sb")
        bk_sb = res.tile([128, MQ], F32, tag="bk_sb", name="bk_sb")
        # bias broadcast along the q/s dim, so paired [128, 2*SC] projection
        # evictions can add bias with a single tensor_tensor. The tiny bias
        # DMAs are emitted here so they queue behind the startup-critical
        # xc0/wq transfers... (they are only consumed ~15us in)
        bq_big = res.tile([128, MQ, SC], F32, tag="bq_big", name="bq_big")
        bk_big = res.tile([128, MQ, SC], F32, tag="bk_big", name="bk_big")
        nc.sync.dma_start(bq_sb[:], bq.ap().rearrange("(m p) -> p m", p=128))
        nc.sync.dma_start(bk_sb[:], bk.ap().rearrange("(m p) -> p m", p=128))
        for big, sb_t in ((bq_big, bq_sb), (bk_big, bk_sb)):
            nc.gpsimd.memset(big[:], 0.0)
            for m in range(MQ):
                nc.vector.tensor_scalar_add(big[:, m, :], big[:, m, :], sb_t[:, m : m + 1])

        # paired additive causal masks: tile jp covers kb-pair blocks
        # (2jp, 2jp+1) of the diagonal 512-chunk; half i keeps
        # k_local <= q_local - 128*(2jp+i)
        masks = []
        for jp in range(NJ // 2):
            mj = res.tile([128, 2 * SC], F32, tag=f"mask{jp}", name=f"mask{jp}")
            nc.gpsimd.memset(mj[:], 0.0)
            for i in range(2):
                nc.gpsimd.affine_select(
                    out=mj[:, i * SC : (i + 1) * SC],
                    in_=mj[:, i * SC : (i + 1) * SC],
                    pattern=[[1, SC]],
                    compare_op=ALU.is_ge,
                    fill=-30000.0,
                    base=-128 * (2 * jp + i),
                    channel_multiplier=-1,
                )
            masks.append(mj)

        for _rep in rep_range:
            # ---------------- phase 1: projections ----------------
            with ExitStack() as p1:
                wpool = p1.enter_context(tc.tile_pool(name="wpool", bufs=1))
                xpool = p1.enter_context(tc.tile_pool(name="xpool", bufs=2))
                pps = p1.enter_context(tc.tile_pool(name="pps", bufs=2, space="PSUM"))

                xT_r = xT.ap().rearrange("(ko p) s -> p ko s", p=128)

                # first x chunk before the weights so the first projection matmul
                # only waits for one k-piece of each; all loads split per
                # k-subtile so compute can start as pieces land
                xc0 = xpool.tile([128, KO, SC], F32R, tag="xc", name="xc")
                for k in range(KO):
                    nc.sync.dma_start(xc0[:, k, :], xT_r[:, k, 0:SC])

                # wq split per-k (gates the first matmuls); wk/wv whole (consumed
                # a few us later, their single transfers finish in time)
                wq_sb = wpool.tile([128, KO, DG], F32R, tag="wq_sb", name="wq_sb")
                wq_r = wq.ap().rearrange("(ko p) n -> p ko n", p=128)
                for k in range(KO):
                    nc.sync.dma_start(wq_sb[:, k, :], wq_r[:, k, :])
                wk_sb = wpool.tile([128, KO, DG], F32R, tag="wk_sb", name="wk_sb")
                nc.sync.dma_start(wk_sb[:], wk.ap().rearrange("(ko p) n -> p ko n", p=128))
                wv_sb = wpool.tile([128, KO, DG], F32R, tag="wv_sb", name="wv_sb")

                def v_groups(xc_v, sc_v):
                    for sb in range(NJ):
                        pv = pps.tile([128, DG], F32, tag="pv", name="pv", bufs=2)
                        for k in range(KO):
                            nc.tensor.matmul(
                                pv[:],
                                xc_v[:, k, sb * 128 : (sb + 1) * 128],
                                wv_sb[:, k, :],
                                start=(k == 0),
                                stop=(k == KO - 1),
                            )
                        blk = sc_v * NJ + sb
                        nc.scalar.activation(
                            vn[:, blk, :, 0:DH],
                            pv[:].rearrange("p (h d) -> p h d", d=DH),
                            AF.Copy,
                        )

                # v(sc) is deferred into iteration sc+1: during the DMA-limited
                # ramp the wv load can then trail wq/wk/xc without stalling PE
                pending_v = None
                for sc in range(NSC):
                    if sc == 0:
                        xc = xc0
                    else:
                        xc = xpool.tile([128, KO, SC], F32R, tag="xc", name="xc")
                        nc.sync.dma_start(xc[:], xT_r[:, :, sc * SC : (sc + 1) * SC])
                    if sc == 1:
                        nc.sync.dma_start(
                            wv_sb[:], wv.ap().rearrange("(ko p) n -> p ko n", p=128)
                        )
                    ssl = slice(sc * SC, (sc + 1) * SC)
                    # qT / kT chunks; two m-subtiles share one 2-bank psum tile so
                    # one DVE op evicts both (with broadcast bias add)
                    for dst, w, b in ((qT, wq_sb, bq_big), (kT, wk_sb, bk_big)):
                        for mp in range(MQ // 2):
                            pq = pps.tile([128, 2, SC], F32, tag="pq", name="pq", bufs=3)
                            for i in range(2):
                                m = 2 * mp + i
                                for k in range(KO):
                                    nc.tensor.matmul(
                                        pq[:, i, :],
                                        w[:, k, m * 128 : (m + 1) * 128],
                                        xc[:, k, :],
                                        start=(k == 0),
                                        stop=(k == KO - 1),
                                    )
                            nc.vector.tensor_tensor(
                                dst[:, 2 * mp : 2 * mp + 2, ssl],
                                pq[:],
                                b[:, 2 * mp : 2 * mp + 2, :],
                                ALU.add,
                            )
                    if pending_v is not None:
                        v_groups(*pending_v)
                    pending_v = (xc, sc)
                v_groups(*pending_v)

            # ---------------- phase 2: attention ----------------
            with ExitStack() as p2:
                wop = p2.enter_context(tc.tile_pool(name="wop", bufs=1))
                ppool = p2.enter_context(tc.tile_pool(name="ppool", bufs=8))
                otsb = p2.enter_context(tc.tile_pool(name="otsb", bufs=2))
                wrk = p2.enter_context(tc.tile_pool(name="wrk", bufs=2))
                outp = p2.enter_context(tc.tile_pool(name="outp", bufs=3))

                wo_sb = wop.tile([128, MQ, D], F32R, tag="wo_sb", name="wo_sb")
                nc.sync.dma_start(wo_sb[:], wo.ap().rearrange("(m p) n -> p m n", p=128))

                with ExitStack() as pa:
                    ps2 = pa.enter_context(tc.tile_pool(name="ps2", bufs=1, space="PSUM"))
                    for qc in range(NSC):
                        npair = (qc + 1) * NJ // 2
                        nkb = npair * 2
                        otc = otsb.tile([128, MQ, SC], F32R, tag="otc", name="otc")
                        qsl = slice(qc * SC, (qc + 1) * SC)
                        for m in range(MQ):
                            # heads a=2m (partitions 0:64) and b=2m+1 (64:128)
                            # processed together: their K=64 score matmuls hit
                            # disjoint PE row groups and run concurrently.
                            ot_a = ps2.tile(
                                [DH + 1, SC], F32, tag="otbc", name="ot_a", bufs=3
                            )
                            ot_b = ps2.tile(
                                [DH + 1, SC], F32, tag="otbc", name="ot_b", bufs=3
                            )

                            def emit_av(ent):
                                kb0, pa_t, pb_t = ent
                                for i in range(2):
                                    kb = kb0 + i
                                    qs = 128 * max(0, kb - qc * NJ)
                                    if qs not in (128, 256):
                                        qs = 0
                                    psl = slice(i * SC + qs, (i + 1) * SC)
                                    osl = slice(qs, SC)
                                    nc.tensor.matmul(
                                        ot_a[:, osl],
                                        vn[:, kb, 2 * m, :],
                                        pa_t[:, psl],
                                        start=(kb == 0),
                                        stop=(kb == nkb - 1),
                                    )
                                    nc.tensor.matmul(
                                        ot_b[:, osl],
                                        vn[:, kb, 2 * m + 1, :],
                                        pb_t[:, psl],
                                        start=(kb == 0),
                                        stop=(kb == nkb - 1),
                                    )

                            pend = []
                            for p in range(npair):
                                kb0 = 2 * p
                                st_a = ps2.tile(
                                    [128, 2 * SC], F32, tag="st", name="st_a", bufs=2
                                )
                                st_b = ps2.tile(
                                    [128, 2 * SC], F32, tag="st", name="st_b", bufs=2
                                )
                                for i in range(2):
                                    kb = kb0 + i
                                    ksl = slice(kb * 128, (kb + 1) * 128)
                                    # causally-valid q starts at 128*(kb-qc*NJ)
                                    # slice when fp32r keeps full rate (N>=256)
                                    qs = 128 * max(0, kb - qc * NJ)
                                    if qs not in (128, 256):
                                        qs = 0
                                    psl = slice(i * SC + qs, (i + 1) * SC)
                                    qvl = slice(qc * SC + qs, (qc + 1) * SC)
                                    nc.tensor.matmul(
                                        st_a[:, psl],
                                        kT[0:64, m, ksl],
                                        qT[0:64, m, qvl],
                                        start=True,
                                        stop=True,
                                    )
                                    nc.tensor.matmul(
                                        st_b[:, psl],
                                        kT[64:128, m, ksl],
                                        qT[64:128, m, qvl],
                                        start=True,
                                        stop=True,
                                    )
                                jp = p - qc * NJ // 2
                                if jp >= 0:
                                    nc.vector.tensor_tensor(
                                        st_a[:], st_a[:], masks[jp][:], ALU.add
                                    )
                                    nc.vector.tensor_tensor(
                                        st_b[:], st_b[:], masks[jp][:], ALU.add
                                    )
                                pa_t = ppool.tile([128, 2 * SC], F32R, tag="pt", name="pa_t")
                                nc.scalar.activation(pa_t[:], st_a[:], AF.Exp, scale=scale)
                                pb_t = ppool.tile([128, 2 * SC], F32R, tag="pt", name="pb_t")
                                nc.scalar.activation(pb_t[:], st_b[:], AF.Exp, scale=scale)
                                pend.append((kb0, pa_t, pb_t))
                                if len(pend) > 3:
                                    emit_av(pend.pop(0))
                            for ent in pend:
                                emit_av(ent)
                            # normalize both heads, stage-interleaved so each
                            # engine works one head while the other completes
                            r_rows, bcs, r64s = [], [], []
                            for ot_ps in (ot_a, ot_b):
                                r_row = wrk.tile([1, SC], F32R, tag="r_row", name="r_row")
                                with nc.allow_low_precision(
                                    reason="fp32r reciprocal for broadcast matmul"
                                ):
                                    nc.vector.reciprocal(r_row[:], ot_ps[DH : DH + 1, :])
                                r_rows.append(r_row)
                            for r_row in r_rows:
                                bc = ps2.tile([64, SC], F32, tag="otbc", name="bc", bufs=3)
                                nc.tensor.matmul(
                                    bc[:], ones64[:], r_row[:], start=True, stop=True
                                )
                                bcs.append(bc)
                            for bc in bcs:
                                r64 = wrk.tile([64, SC], F32, tag="r64", name="r64")
                                nc.vector.tensor_copy(r64[:], bc[:])
                                r64s.append(r64)
                            for hb in range(2):
                                nc.vector.tensor_tensor(
                                    otc[64 * hb : 64 * hb + 64, m, :],
                                    (ot_a, ot_b)[hb][0:DH, :],
                                    r64s[hb][:],
                                    ALU.mult,
                                )

                        # out-projection for this q-chunk, interleaved so the
                        # stores overlap the remaining attention compute
                        for n in range(NO):
                            for sb in range(NJ):
                                po = ps2.tile([128, 512], F32, tag="po", name="po", bufs=1)
                                for g in range(MQ):
                                    nc.tensor.matmul(
                                        po[:],
                                        otc[:, g, sb * 128 : (sb + 1) * 128],
                                        wo_sb[:, g, n * 512 : (n + 1) * 512],
                                        start=(g == 0),
                                        stop=(g == MQ - 1),
                                    )
                                outt = outp.tile([128, 512], F32, tag="outt", name="outt")
                                nc.vector.tensor_copy(outt[:], po[:])
                                r0 = qc * SC + sb * 128
                                nc.sync.dma_start(
                                    out.ap()[r0 : r0 + 128, n * 512 : (n + 1) * 512],
                                    outt[:],
                                )

    nc.compile()
    return nc


REPS = 1  # >1 only for device-time measurement via wall-clock deltas


def _get_nc(SEQ, D, DG, HPG):
    key = (SEQ, D, DG, HPG, REPS)
    if key not in _cache:
        _cache[key] = _build(SEQ, D, DG, HPG, REPS)
    return _cache[key]


def _r22(a):
    """Truncate fp32 mantissa to 13 bits (FP22 / fp32r operand format)."""
    v = np.ascontiguousarray(a, dtype=np.float32).view(np.uint32)
    return (v & np.uint32(0xFFFFFC00)).view(np.float32)


def kernel(x, wq, bq, wk, bk, wv, bv, wo, bo):
    from concourse.bass_utils import run_bass_kernel_spmd

    x = np.asarray(x, dtype=np.float32)
    wq = np.asarray(wq, dtype=np.float32)
    wk = np.asarray(wk, dtype=np.float32)
    wv = np.asarray(wv, dtype=np.float32)
    wo = np.asarray(wo, dtype=np.float32)
    bq = np.asarray(bq, dtype=np.float32)
    bk = np.asarray(bk, dtype=np.float32)
    bv = np.asarray(bv, dtype=np.float32)
    bo = np.asarray(bo, dtype=np.float32)

    bsz, SEQ, D = x.shape
    DG = D // TP
    HPG = N_HEADS // TP
    assert bsz * TP == N_CORES

    nc = _get_nc(SEQ, D, DG, HPG)

    in_maps = []
    for c in range(N_CORES):
        b, g = c // TP, c % TP
        csl = slice(g * DG, (g + 1) * DG)
        in_maps.append(
            {
                "xT": _r22(x[b].T),
                "wq": _r22(wq[:, csl]),
                "wk": _r22(wk[:, csl]),
                "wv": _r22(wv[:, csl]),
                "wo": _r22(wo[csl, :]),
                "bq": np.ascontiguousarray(bq[csl]),
                "bk": np.ascontiguousarray(bk[csl]),
            }
        )

    global last_result
    res = None
    for attempt in range(3):
        try:
            res = run_bass_kernel_spmd(
                nc, in_maps, core_ids=list(range(N_CORES)), trace=TRACE
            )
            break
        except Exception:
            # transient device errors (NRT_EXEC_UNIT_UNRECOVERABLE) appear when
            # a previous process's teardown races our startup; they clear after
            # a short recovery delay
            if attempt == 2:
                raise
            import time as _time

            _time.sleep(15)
    assert res is not None
    last_result = res

    # host combine: sum the TP partials, add bias terms (bv @ wo + bo)
    bias = (bv @ wo + bo).astype(np.float32)
    outs = np.empty((bsz, SEQ, D), dtype=np.float32)
    for b in range(bsz):
        acc = res.results[b * TP]["out"].astype(np.float32).copy()
        for g in range(1, TP):
            acc += res.results[b * TP + g]["out"]
        outs[b] = acc + bias[None, :]
    return outs



# revision 2
# speedup vs baseline: 1.1635x; 1.1635x over previous
"""Causal self-attention Trainium2 Bass kernel (fp8 DoubleRow pipeline).

Problem (hardcoded): x [4, 2048, 1024] f32, wq/wk/wv/wo [1024, 1024], biases
[1024]; out = causal_mha(x) @ wo + bo with 16 heads of dim 64.

Sharding over 8 NeuronCores: data parallel on batch (4) x tensor parallel on
heads (2 groups of 8 heads). Core c handles batch c//2 and head-group c%2.
Each core computes its partial out-projection; the host sums the two partials
per batch, divides by the operand prescale (32*32=1024) and adds the bias
terms (bo + bv @ wo).

Numerics: weights are prescaled x32 on the host and split into fp8e4m3
hi/lo pairs (w = (w_hi + w_lo)/32, error ~0.1%); x is split the same way.
All projection matmuls run in fp8 DoubleRow perf mode (2 contraction planes
per instruction at 0.5 cycles/row = 4x fp32r throughput) with 3 product
terms (hi*hi + lo*hi + hi*lo). q/k are evicted to fp8 for the score matmuls
(chunks >=1), and to fp32r for the q-chunk-0 scores where few-entry softmax
rows make logit noise expensive. v is evicted as an fp8 hi/lo pair stored as
the two DoubleRow planes of the AV matmul (against a stride-0 broadcast P),
which makes the 2-term v reconstruction free. Scores for chunks >=1 use
DoubleRow with both planes stride-0 aliased to the same q/k data (exact 2x,
absorbed into the exp scale). P = exp(scale*s - 1) is written by the
activation engine directly as fp8e4m3 (max logit ~6.1 keeps exp(s-1) < 240),
causally masked in-place by gpsimd affine_select on the diagonal blocks, and
the ones-column of the v-hi plane accumulates the softmax normalizer. The
out-projection runs in bf16 (otc and wo), and out tiles stream to HBM in f32.
"""

import numpy as np

N_HEADS = 16
DH = 64
N_CORES = 8
TP = 2  # head groups
WS = 32.0  # host-side weight prescale

_cache = {}
TRACE = False  # set by test harness to request an NTFF trace
last_result = None  # BassKernelResults of the most recent kernel() call


def _build(SEQ, D, DG, HPG, reps=1):
    """Build + schedule the per-core Bass program. DG = per-core qkv width,
    HPG = heads per core."""
    from contextlib import ExitStack

    import concourse.tile as tile
    from concourse import bacc, mybir

    F32 = mybir.dt.float32
    F32R = mybir.dt.float32r
    BF16 = mybir.dt.bfloat16
    F8 = mybir.dt.float8e4
    AF = mybir.ActivationFunctionType
    ALU = mybir.AluOpType
    DR = mybir.MatmulPerfMode.DoubleRow

    KO = D // 128  # contraction subtiles for the projections
    KP = KO // 2  # DoubleRow contraction pairs
    MQ = DG // 128  # qkv-dim subtiles
    SC = 512  # q/s chunk size
    NSC = SEQ // SC  # chunks
    NJ = SC // 128  # 128-blocks per chunk
    NSB = SEQ // 128  # s blocks total
    NO = D // 512  # out-proj column chunks
    VW = 68  # padded v row: [v0..v63, ones, pad, pad, pad]

    nc = bacc.Bacc("TRN2", target_bir_lowering=False, debug=False)
    xh = nc.dram_tensor("xh", [D, SEQ], F8, kind="ExternalInput")
    xl = nc.dram_tensor("xl", [D, SEQ], F8, kind="ExternalInput")
    wqh = nc.dram_tensor("wqh", [D, DG], F8, kind="ExternalInput")
    wql = nc.dram_tensor("wql", [D, DG], F8, kind="ExternalInput")
    wkh = nc.dram_tensor("wkh", [D, DG], F8, kind="ExternalInput")
    wkl = nc.dram_tensor("wkl", [D, DG], F8, kind="ExternalInput")
    wvh = nc.dram_tensor("wvh", [D, DG], F8, kind="ExternalInput")
    wvl = nc.dram_tensor("wvl", [D, DG], F8, kind="ExternalInput")
    wob = nc.dram_tensor("wob", [DG, D], BF16, kind="ExternalInput")
    bq = nc.dram_tensor("bq", [DG], F32, kind="ExternalInput")
    bk = nc.dram_tensor("bk", [DG], F32, kind="ExternalInput")
    out = nc.dram_tensor("out", [SEQ, D], F32, kind="ExternalOutput")

    # psum score value = 2 (stride-0 alias) * WS^2 (operand scales) * 8 * s
    scale_fp8 = 1.0 / (2 * WS * WS * np.sqrt(DH))
    scale_f32r = 1.0 / (WS * WS * np.sqrt(DH))
    C_SHIFT = 1.0

    rep_range = range(reps)
    with tile.TileContext(nc) as tc, ExitStack() as ctx:
        # pools alive for the whole kernel
        res = ctx.enter_context(tc.tile_pool(name="res", bufs=1))
        qT8 = res.tile([128, MQ, SEQ], F8, tag="qT8", name="qT8")
        kT8 = res.tile([128, MQ, SEQ], F8, tag="kT8", name="kT8")
        qT0 = res.tile([128, MQ, SC], F32R, tag="qT0", name="qT0")
        kT0 = res.tile([128, MQ, SC], F32R, tag="kT0", name="kT0")
        vn = res.tile([128, NSB, HPG, 2, VW], F8, tag="vn", name="vn")

        ones64 = res.tile([1, 64], F32R, tag="ones64", name="ones64")
        ones64_f = res.tile([1, 64], F32, tag="ones64_f", name="ones64_f")
        nc.gpsimd.memset(ones64_f[:], 1.0)
        nc.vector.tensor_copy(ones64[:], ones64_f[:])
        # ones / zeros columns of the v planes (softmax normalizer)
        ones_nb = res.tile([128, NSB, HPG], F32, tag="ones_nb", name="ones_nb")
        nc.gpsimd.memset(ones_nb[:], 1.0)
        nc.vector.tensor_copy(vn[:, :, :, 0, DH], ones_nb[:])
        nc.gpsimd.memset(ones_nb[:], 0.0)
        nc.vector.tensor_copy(vn[:, :, :, 1, DH], ones_nb[:])

        cbias = res.tile([128, 1], F32, tag="cbias", name="cbias")
        nc.gpsimd.memset(cbias[:], -C_SHIFT)

        bq_sb = res.tile([128, MQ], F32, tag="bq_sb", name="bq_sb")
        bk_sb = res.tile([128, MQ], F32, tag="bk_sb", name="bk_sb")
        # bias broadcast along the q/s dim so paired [128, 2*SC] projection
        # evictions can add bias with a single tensor_tensor
        bq_big = res.tile([128, MQ, SC], F32, tag="bq_big", name="bq_big")
        bk_big = res.tile([128, MQ, SC], F32, tag="bk_big", name="bk_big")
        nc.sync.dma_start(bq_sb[:], bq.ap().rearrange("(m p) -> p m", p=128))
        nc.sync.dma_start(bk_sb[:], bk.ap().rearrange("(m p) -> p m", p=128))
        for big, sb_t in ((bq_big, bq_sb), (bk_big, bk_sb)):
            nc.gpsimd.memset(big[:], 0.0)
            for m in range(MQ):
                nc.vector.tensor_scalar_add(big[:, m, :], big[:, m, :], sb_t[:, m : m + 1])

        for _rep in rep_range:
            # ---------------- phase 1: projections ----------------
            with ExitStack() as p1:
                wpool = p1.enter_context(tc.tile_pool(name="wpool", bufs=1))
                xpool = p1.enter_context(tc.tile_pool(name="xpool", bufs=2))
                pps = p1.enter_context(tc.tile_pool(name="pps", bufs=2, space="PSUM"))

                xh_r = xh.ap().rearrange("(ko p) s -> p ko s", p=128)
                xl_r = xl.ap().rearrange("(ko p) s -> p ko s", p=128)

                # first x_hi chunk + per-k wq_hi gate the first matmuls
                xc0h = xpool.tile([128, KO, SC], F8, tag="xch", name="xch")
                for k in range(KO):
                    nc.sync.dma_start(xc0h[:, k, :], xh_r[:, k, 0:SC])
                wq_h = wpool.tile([128, KO, DG], F8, tag="wq_h", name="wq_h")
                wq_r = wqh.ap().rearrange("(ko p) n -> p ko n", p=128)
                for k in range(KO):
                    nc.sync.dma_start(wq_h[:, k, :], wq_r[:, k, :])
                xc0l = xpool.tile([128, KO, SC], F8, tag="xcl", name="xcl")
                for k in range(KO):
                    nc.sync.dma_start(xc0l[:, k, :], xl_r[:, k, 0:SC])
                wq_l = wpool.tile([128, KO, DG], F8, tag="wq_l", name="wq_l")
                nc.sync.dma_start(wq_l[:], wql.ap().rearrange("(ko p) n -> p ko n", p=128))
                wk_h = wpool.tile([128, KO, DG], F8, tag="wk_h", name="wk_h")
                nc.sync.dma_start(wk_h[:], wkh.ap().rearrange("(ko p) n -> p ko n", p=128))
                wk_l = wpool.tile([128, KO, DG], F8, tag="wk_l", name="wk_l")
                nc.sync.dma_start(wk_l[:], wkl.ap().rearrange("(ko p) n -> p ko n", p=128))
                wv_h = wpool.tile([128, KO, DG], F8, tag="wv_h", name="wv_h")
                wv_l = wpool.tile([128, KO, DG], F8, tag="wv_l", name="wv_l")

                def v_groups(xch_v, xcl_v, sc_v):
                    for sb in range(NJ):
                        pv = pps.tile([128, DG], F32, tag="pv", name="pv", bufs=2)
                        ssl = slice(sb * 128, (sb + 1) * 128)
                        terms = ((xch_v, wv_h), (xcl_v, wv_h), (xch_v, wv_l))
                        nt = len(terms)
                        for t, (xa, wa) in enumerate(terms):
                            for kp in range(KP):
                                nc.tensor.matmul(
                                    pv[:],
                                    xa[:, 2 * kp : 2 * kp + 2, ssl],
                                    wa[:, 2 * kp : 2 * kp + 2, :],
                                    start=(t == 0 and kp == 0),
                                    stop=(t == nt - 1 and kp == KP - 1),
                                    perf_mode=DR,
                                )
                        blk = sc_v * NJ + sb
                        pv_r = pv[:].rearrange("p (h d) -> p h d", d=DH)
                        # v hi plane then lo = pv - hi (2-term v at fp8)
                        nc.vector.tensor_copy(vn[:, blk, :, 0, 0:DH], pv_r)
                        nc.vector.tensor_tensor(
                            vn[:, blk, :, 1, 0:DH],
                            pv_r,
                            vn[:, blk, :, 0, 0:DH],
                            ALU.subtract,
                        )

                pending_v = None
                for sc in range(NSC):
                    if sc == 0:
                        xch, xcl = xc0h, xc0l
                    else:
                        xch = xpool.tile([128, KO, SC], F8, tag="xch", name="xch")
                        nc.sync.dma_start(xch[:], xh_r[:, :, sc * SC : (sc + 1) * SC])
                        xcl = xpool.tile([128, KO, SC], F8, tag="xcl", name="xcl")
                        nc.sync.dma_start(xcl[:], xl_r[:, :, sc * SC : (sc + 1) * SC])
                    if sc == 1:
                        nc.sync.dma_start(
                            wv_h[:], wvh.ap().rearrange("(ko p) n -> p ko n", p=128)
                        )
                        nc.sync.dma_start(
                            wv_l[:], wvl.ap().rearrange("(ko p) n -> p ko n", p=128)
                        )
                    ssl = slice(sc * SC, (sc + 1) * SC)
                    for dst, wh_t, wl_t, b, dst0 in (
                        (qT8, wq_h, wq_l, bq_big, qT0),
                        (kT8, wk_h, wk_l, bk_big, kT0),
                    ):
                        for mp in range(MQ // 2):
                            pq = pps.tile([128, 2, SC], F32, tag="pq", name="pq", bufs=2)
                            for i in range(2):
                                m = 2 * mp + i
                                msl = slice(m * 128, (m + 1) * 128)
                                terms = ((xch, wh_t), (xcl, wh_t), (xch, wl_t))
                                nt = len(terms)
                                for t, (xa, wa) in enumerate(terms):
                                    for kp in range(KP):
                                        nc.tensor.matmul(
                                            pq[:, i, :],
                                            wa[:, 2 * kp : 2 * kp + 2, msl],
                                            xa[:, 2 * kp : 2 * kp + 2, :],
                                            start=(t == 0 and kp == 0),
                                            stop=(t == nt - 1 and kp == KP - 1),
                                            perf_mode=DR,
                                        )
                            msl2 = slice(2 * mp, 2 * mp + 2)
                            nc.vector.tensor_tensor(
                                dst[:, msl2, ssl], pq[:], b[:, msl2, :], ALU.add
                            )
                            if sc == 0:
                                # f32r copy of chunk 0 for the qc=0 scores
                                nc.vector.tensor_tensor(
                                    dst0[:, msl2, :], pq[:], b[:, msl2, :], ALU.add
                                )
                    if pending_v is not None:
                        v_groups(*pending_v)
                    pending_v = (xch, xcl, sc)
                v_groups(*pending_v)

            # ---------------- phase 2: attention ----------------
            with ExitStack() as p2:
                wop = p2.enter_context(tc.tile_pool(name="wop", bufs=1))
                ppool = p2.enter_context(tc.tile_pool(name="ppool", bufs=8))
                otsb = p2.enter_context(tc.tile_pool(name="otsb", bufs=2))
                wrk = p2.enter_context(tc.tile_pool(name="wrk", bufs=2))
                outp = p2.enter_context(tc.tile_pool(name="outp", bufs=3))

                wo_sb = wop.tile([128, MQ, D], BF16, tag="wo_sb", name="wo_sb")
                nc.sync.dma_start(wo_sb[:], wob.ap().rearrange("(m p) n -> p m n", p=128))

                with ExitStack() as pa_ctx:
                    ps2 = pa_ctx.enter_context(
                        tc.tile_pool(name="ps2", bufs=1, space="PSUM")
                    )
                    for qc in range(NSC):
                        npair = (qc + 1) * NJ // 2
                        nkb = npair * 2
                        escale = scale_f32r if qc == 0 else scale_fp8
                        otc = otsb.tile([128, MQ, SC], BF16, tag="otc", name="otc")
                        for m in range(MQ):
                            # heads a=2m (partitions 0:64) and b=2m+1 (64:128)
                            ot_a = ps2.tile([DH + 1, SC], F32, tag="otbc", name="ot_a", bufs=3)
                            ot_b = ps2.tile([DH + 1, SC], F32, tag="otbc", name="ot_b", bufs=3)

                            def emit_av(ent):
                                kb0, pa_t, pb_t = ent
                                for i in range(2):
                                    kb = kb0 + i
                                    qs = 128 * max(0, kb - qc * NJ)
                                    osl = slice(qs, SC)
                                    w = SC - qs
                                    for hb, pt in ((0, pa_t), (1, pb_t)):
                                        nc.tensor.matmul(
                                            (ot_a, ot_b)[hb][:, osl],
                                            vn[:, kb, 2 * m + hb, :, 0 : DH + 1],
                                            pt[:, i, osl]
                                            .unsqueeze(1)
                                            .broadcast_to([128, 2, w]),
                                            start=(kb == 0),
                                            stop=(kb == nkb - 1),
                                            perf_mode=DR,
                                        )

                            pend = []
                            for p in range(npair):
                                kb0 = 2 * p
                                qs_p = 128 * max(0, kb0 - qc * NJ)
                                wp = SC - qs_p
                                psl = slice(qs_p, SC)
                                st_a = ps2.tile([128, 2, SC], F32, tag="st", name="st_a", bufs=2)
                                st_b = ps2.tile([128, 2, SC], F32, tag="st", name="st_b", bufs=2)
                                for i in range(2):
                                    kb = kb0 + i
                                    ksl = slice(kb * 128, (kb + 1) * 128)
                                    qvl = slice(qc * SC + qs_p, (qc + 1) * SC)
                                    for hb, st in ((0, st_a), (1, st_b)):
                                        hsl = slice(64 * hb, 64 * hb + 64)
                                        if qc == 0:
                                            nc.tensor.matmul(
                                                st[:, i, psl],
                                                kT0[hsl, m, ksl],
                                                qT0[hsl, m, qs_p:SC],
                                                start=True,
                                                stop=True,
                                            )
                                        else:
                                            nc.tensor.matmul(
                                                st[:, i, psl],
                                                kT8[hsl, m, ksl]
                                                .unsqueeze(1)
                                                .broadcast_to([64, 2, 128]),
                                                qT8[hsl, m, qvl]
                                                .unsqueeze(1)
                                                .broadcast_to([64, 2, wp]),
                                                start=True,
                                                stop=True,
                                                perf_mode=DR,
                                            )
                                pa_t = ppool.tile([128, 2, SC], F8, tag="pt", name="pa_t")
                                nc.scalar.activation(
                                    pa_t[:, :, psl], st_a[:, :, psl], AF.Exp,
                                    scale=escale, bias=cbias[:],
                                )
                                pb_t = ppool.tile([128, 2, SC], F8, tag="pt", name="pb_t")
                                nc.scalar.activation(
                                    pb_t[:, :, psl], st_b[:, :, psl], AF.Exp,
                                    scale=escale, bias=cbias[:],
                                )
                                # zero causally-invalid P on diagonal-chunk blocks
                                for i in range(2):
                                    kb = kb0 + i
                                    j = kb - qc * NJ
                                    if j < 0:
                                        continue
                                    for pt in (pa_t, pb_t):
                                        nc.gpsimd.affine_select(
                                            out=pt[:, i, psl],
                                            in_=pt[:, i, psl],
                                            pattern=[[1, wp]],
                                            compare_op=ALU.is_ge,
                                            fill=0.0,
                                            base=qs_p - 128 * j,
                                            channel_multiplier=-1,
                                        )
                                pend.append((kb0, pa_t, pb_t))
                                if len(pend) > 3:
                                    emit_av(pend.pop(0))
                            for ent in pend:
                                emit_av(ent)
                            # normalize both heads, stage-interleaved
                            r_rows, bcs, r64s = [], [], []
                            for ot_ps in (ot_a, ot_b):
                                r_row = wrk.tile([1, SC], F32R, tag="r_row", name="r_row")
                                with nc.allow_low_precision(
                                    reason="fp32r reciprocal for broadcast matmul"
                                ):
                                    nc.vector.reciprocal(r_row[:], ot_ps[DH : DH + 1, :])
                                r_rows.append(r_row)
                            for r_row in r_rows:
                                bc = ps2.tile([64, SC], F32, tag="otbc", name="bc", bufs=3)
                                nc.tensor.matmul(
                                    bc[:], ones64[:], r_row[:], start=True, stop=True
                                )
                                bcs.append(bc)
                            for bc in bcs:
                                r64 = wrk.tile([64, SC], F32, tag="r64", name="r64")
                                nc.vector.tensor_copy(r64[:], bc[:])
                                r64s.append(r64)
                            for hb in range(2):
                                nc.vector.tensor_tensor(
                                    otc[64 * hb : 64 * hb + 64, m, :],
                                    (ot_a, ot_b)[hb][0:DH, :],
                                    r64s[hb][:],
                                    ALU.mult,
                                )

                        # out-projection for this q-chunk (bf16 operands)
                        for n in range(NO):
                            for sb in range(NJ):
                                po = ps2.tile([128, 512], F32, tag="po", name="po", bufs=1)
                                for g in range(MQ):
                                    nc.tensor.matmul(
                                        po[:],
                                        otc[:, g, sb * 128 : (sb + 1) * 128],
                                        wo_sb[:, g, n * 512 : (n + 1) * 512],
                                        start=(g == 0),
                                        stop=(g == MQ - 1),
                                    )
                                outt = outp.tile([128, 512], F32, tag="outt", name="outt")
                                nc.vector.tensor_copy(outt[:], po[:])
                                r0 = qc * SC + sb * 128
                                nc.sync.dma_start(
                                    out.ap()[r0 : r0 + 128, n * 512 : (n + 1) * 512],
                                    outt[:],
                                )

    nc.compile()
    return nc


REPS = 1  # >1 only for device-time measurement via wall-clock deltas


def _get_nc(SEQ, D, DG, HPG):
    key = (SEQ, D, DG, HPG, REPS)
    if key not in _cache:
        _cache[key] = _build(SEQ, D, DG, HPG, REPS)
    return _cache[key]


def _split8(a):
    """fp8e4m3 hi/lo split of an f32 array."""
    import ml_dtypes

    E4 = ml_dtypes.float8_e4m3
    a = np.ascontiguousarray(a, dtype=np.float32)
    hi = a.astype(E4)
    lo = (a - hi.astype(np.float32)).astype(E4)
    return hi, lo


def kernel(x, wq, bq, wk, bk, wv, bv, wo, bo):
    import ml_dtypes
    from concourse.bass_utils import run_bass_kernel_spmd

    BF = ml_dtypes.bfloat16

    x = np.asarray(x, dtype=np.float32)
    wq = np.asarray(wq, dtype=np.float32)
    wk = np.asarray(wk, dtype=np.float32)
    wv = np.asarray(wv, dtype=np.float32)
    wo = np.asarray(wo, dtype=np.float32)
    bq = np.asarray(bq, dtype=np.float32)
    bk = np.asarray(bk, dtype=np.float32)
    bv = np.asarray(bv, dtype=np.float32)
    bo = np.asarray(bo, dtype=np.float32)

    bsz, SEQ, D = x.shape
    DG = D // TP
    HPG = N_HEADS // TP
    assert bsz * TP == N_CORES

    nc = _get_nc(SEQ, D, DG, HPG)

    xs = [_split8(x[b].T) for b in range(bsz)]
    in_maps = []
    for c in range(N_CORES):
        b, g = c // TP, c % TP
        csl = slice(g * DG, (g + 1) * DG)
        wq_h, wq_l = _split8(WS * wq[:, csl])
        wk_h, wk_l = _split8(WS * wk[:, csl])
        wv_h, wv_l = _split8(WS * wv[:, csl])
        in_maps.append(
            {
                "xh": xs[b][0],
                "xl": xs[b][1],
                "wqh": wq_h,
                "wql": wq_l,
                "wkh": wk_h,
                "wkl": wk_l,
                "wvh": wv_h,
                "wvl": wv_l,
                "wob": np.ascontiguousarray(WS * wo[csl, :]).astype(BF),
                "bq": np.ascontiguousarray(WS * bq[csl]),
                "bk": np.ascontiguousarray(WS * bk[csl]),
            }
        )

    global last_result
    res = None
    for attempt in range(3):
        try:
            res = run_bass_kernel_spmd(
                nc, in_maps, core_ids=list(range(N_CORES)), trace=TRACE
            )
            break
        except Exception:
            # transient device errors (NRT_EXEC_UNIT_UNRECOVERABLE) appear when
            # a previous process's teardown races our startup; they clear after
            # a short recovery delay
            if attempt == 2:
                raise
            import time as _time

            _time.sleep(15)
    assert res is not None
    last_result = res

    # host combine: sum the TP partials, undo the x32 weight prescales,
    # add bias terms (bv @ wo + bo)
    bias = (bv @ wo + bo).astype(np.float32)
    outs = np.empty((bsz, SEQ, D), dtype=np.float32)
    inv = 1.0 / (WS * WS)
    for b in range(bsz):
        acc = res.results[b * TP]["out"].astype(np.float32).copy()
        for g in range(1, TP):
            acc += res.results[b * TP + g]["out"]
        outs[b] = acc * inv + bias[None, :]
    return outs
